# revision 1
# baseline (speedup 1.0000x reference)
"""Trainium2 Bass kernel for nn_MCPBRNN_Generic_PETconstraint_Two_VariantOutputGate_BYPASSM0.

Strategy
--------
The model is a scalar-state (H=1) nonlinear recurrence over T=524288 steps.
Every per-step output is a pure elementwise function of the PRE-update state
c[t] and the inputs u1[t], u2[t], so the kernel has two phases:

  Phase A: compute the state trajectory c[t]. The map c -> F(c, t) is a strong
    contraction (|dF/dc| ~ 0.65), so the sequence is split into
    8 cores x 128 partitions x F lanes = 4096 independent chunks of S steps;
    each chunk starts from c=0 and runs W warm-up steps through the preceding
    inputs, which converges the state to fp32 noise (measured: W=32 reaches
    ~5e-7 worst-case; W=40 used for margin).  Per step the update is

      c1 = P(c) - oo1*sig(A1*c+B1)*c - min(ol_t*c, u2_t) - max(expC-u1_t, c) + expC
         = P(c) + y1 + relu(u2_t - ol_t*c) - relu(c - (expC-u1_t)) + (u1_t - u2_t)

    where P(c) = c*(1 - gw1*s2(c)) with the tiny-argument sigmoid s2 replaced
    by a host-fitted quadratic (max fit error ~1e-7, verified at runtime),
    factored as p3*((c+alpha)*c+beta)*c so each step is 3 ACT + 7 DVE + 3 Pool
    instructions.

  Phase B: recompute all 14 outputs from the stored c[t] with fully vectorized
    ops + DMA out.

Inputs are re-laid-out on the host so each SBUF column block [128, F] holds
one time step for 128*F chunks (chunk k <-> (partition p, lane f), k = p*F+f).
"""
import numpy as np

import concourse.bacc as bacc
import concourse.tile as tile
import concourse.mybir as mybir

P = 128
NCORE = 8
T = 524288
F = 32            # free-dim lanes per partition
S = 16            # steps stored per chunk;  P*F*S*NCORE == T
W = 40            # warm-up steps
NSTEP = W + S
COLS = NSTEP * F
CORE_T = P * F * S          # 65536 time steps handled per core
YCOLS = CORE_T // P         # y_obs columns per core (512)
SPIN = 365
TRAIN = 262144

AL = mybir.AluOpType
AF = mybir.ActivationFunctionType
F32 = mybir.dt.float32

OUT_NAMES = ["o_h", "o_c", "o_l", "o_lc", "o_bp", "o_gw",
             "o_ib", "o_oo", "o_ol", "o_olc", "o_f", "o_oogw"]

_cache = {}


def _build_program(sc):
    """sc: dict of python-float scalars baked into the program."""
    nc = bacc.Bacc("TRN2", target_bir_lowering=False, debug=False,
                   num_devices=NCORE)
    u1d = nc.dram_tensor("u1p", [P, COLS], F32, kind="ExternalInput").ap()
    u2d = nc.dram_tensor("u2p", [P, COLS], F32, kind="ExternalInput").ap()
    yd = nc.dram_tensor("yp", [P, YCOLS], F32, kind="ExternalInput").ap()
    outs = {nm: nc.dram_tensor(nm, [P, S * F], F32, kind="ExternalOutput").ap()
            for nm in OUT_NAMES}
    yred = nc.dram_tensor("yred", [P, 2], F32, kind="ExternalOutput").ap()

    with tile.TileContext(nc) as tc:
        with tc.tile_pool(name="main", bufs=1) as pool:
            # persistent buffers
            u1b = pool.tile([P, COLS], F32, name="u1b")
            u2b = pool.tile([P, COLS], F32, name="u2b")
            s3n = pool.tile([P, COLS], F32, name="s3n")   # -ol1*sigmoid(A3*u2+B3)
            qnb = pool.tile([P, COLS], F32, name="qnb")   # u1 - expC
            w12 = pool.tile([P, COLS], F32, name="w12")   # u1 - u2
            cb = pool.tile([P, COLS + F], F32, name="cb")
            # bias constants for ACT (bias must be an AP for non-Copy funcs)
            bB1 = pool.tile([P, 1], F32, name="bB1")
            bB3 = pool.tile([P, 1], F32, name="bB3")
            bB2 = pool.tile([P, 1], F32, name="bB2")
            bz = pool.tile([P, 1], F32, name="bz")
            nc.vector.memset(bB1[:], sc["B1"])
            nc.vector.memset(bB3[:], sc["B3"])
            nc.vector.memset(bB2[:], sc["B2"])
            nc.vector.memset(bz[:], 0.0)

            nc.sync.dma_start(u1b[:], u1d[:, :])
            nc.sync.dma_start(u2b[:], u2d[:, :])

            # big precomputes
            nc.scalar.activation(s3n[:], u2b[:], AF.Sigmoid, bias=bB3[:, 0:1],
                                 scale=sc["A3"])
            nc.vector.tensor_scalar(s3n[:], s3n[:], -sc["ol1"], None, AL.mult)
            nc.vector.tensor_scalar(qnb[:], u1b[:], -sc["expC"], None, AL.add)
            nc.vector.scalar_tensor_tensor(w12[:], u2b[:], -1.0, u1b[:],
                                           AL.mult, AL.add)
            nc.vector.memset(cb[:, 0:F], 0.0)

            # phase A temporaries
            names = ["s1", "e1", "e2", "y1", "t1", "t2", "aam", "p1", "cq",
                     "p2", "r2", "pq"]
            tmp = {nm: pool.tile([P, F], F32, name=f"pa_{nm}") for nm in names}

            for step in range(NSTEP):
                lo, hi = step * F, (step + 1) * F
                c = cb[:, lo:hi]
                s1, e1, e2, y1, t1, t2 = (tmp[n] for n in
                                          ("s1", "e1", "e2", "y1", "t1", "t2"))
                aam, p1, cq, p2, r2, pq = (tmp[n] for n in
                                           ("aam", "p1", "cq", "p2", "r2", "pq"))
                nc.scalar.activation(s1[:], c, AF.Sigmoid, bias=bB1[:, 0:1],
                                     scale=sc["A1"])
                nc.vector.scalar_tensor_tensor(e1[:], c, sc["alpha"], c,
                                               AL.add, AL.mult)
                nc.vector.scalar_tensor_tensor(e2[:], e1[:], sc["beta"], c,
                                               AL.add, AL.mult)
                nc.vector.scalar_tensor_tensor(y1[:], s1[:], -sc["oo1"], c,
                                               AL.mult, AL.mult)
                nc.vector.scalar_tensor_tensor(t1[:], e2[:], sc["p3"], y1[:],
                                               AL.mult, AL.add)
                nc.gpsimd.tensor_tensor(aam[:], s3n[:, lo:hi], c, AL.mult)
                nc.gpsimd.tensor_tensor(p1[:], aam[:], u2b[:, lo:hi], AL.add)
                nc.gpsimd.tensor_tensor(cq[:], c, qnb[:, lo:hi], AL.add)
                nc.scalar.activation(p2[:], p1[:], AF.Relu, bias=bz[:, 0:1],
                                     scale=1.0)
                nc.scalar.activation(r2[:], cq[:], AF.Relu, bias=bz[:, 0:1],
                                     scale=1.0)
                nc.vector.scalar_tensor_tensor(pq[:], r2[:], -1.0, p2[:],
                                               AL.mult, AL.add)
                nc.vector.tensor_tensor(t2[:], t1[:], w12[:, lo:hi], AL.add)
                nc.vector.tensor_tensor(cb[:, hi:hi + F], t2[:], pq[:], AL.add)

            # ---- phase B ----
            st = W * F           # start col of the stored region
            cS = cb[:, st:COLS]
            u1S = u1b[:, st:COLS]
            u2S = u2b[:, st:COLS]
            qnS = qnb[:, st:COLS]
            s3S = s3n[:, st:COLS]
            NB = S * F

            def bt(nm):
                return pool.tile([P, NB], F32, name=f"pb_{nm}")

            s1b = bt("s1b"); s2b = bt("s2b"); oo = bt("oo"); oogw = bt("oogw")
            ol = bt("ol"); pxs = bt("pxs"); px = bt("px"); hn = bt("hn")
            ln = bt("ln"); lcn = bt("lcn"); gwn = bt("gwn")
            mk = bt("mk"); mk1 = bt("mk1"); den = bt("den"); rec = bt("rec")
            ib = bt("ib"); cmk = bt("cmk"); cm1 = bt("cm1"); csf = bt("csf")
            crc = bt("crc"); vv = bt("vv"); mn = bt("mn"); dd = bt("dd")
            olc = bt("olc"); f1 = bt("f1"); ff = bt("ff"); t0 = bt("t0")

            nc.scalar.activation(s1b[:], cS, AF.Sigmoid, bias=bB1[:, 0:1],
                                 scale=sc["A1"])
            nc.scalar.activation(s2b[:], cS, AF.Sigmoid, bias=bB2[:, 0:1],
                                 scale=sc["A2"])
            nc.vector.tensor_scalar(oo[:], s1b[:], sc["oo1"], None, AL.mult)
            nc.vector.tensor_scalar(oogw[:], s2b[:], sc["gw1"], None, AL.mult)
            nc.vector.tensor_scalar(ol[:], s3S, -1.0, None, AL.mult)
            nc.vector.tensor_tensor(pxs[:], cS, qnS, AL.add)
            nc.vector.tensor_scalar(px[:], pxs[:], 0.0, None, AL.max)
            nc.vector.tensor_tensor(t0[:], oo[:], cS, AL.mult)
            nc.vector.tensor_tensor(hn[:], t0[:], px[:], AL.add)
            nc.vector.tensor_tensor(ln[:], ol[:], cS, AL.mult)
            nc.vector.tensor_tensor(lcn[:], ln[:], u2S, AL.min)
            nc.vector.tensor_tensor(gwn[:], oogw[:], cS, AL.mult)
            # ib = where(u1>0, px/u1, 0)
            nc.vector.tensor_scalar(mk[:], u1S, 0.0, None, AL.is_gt)
            nc.vector.tensor_scalar(mk1[:], mk[:], -1.0, 1.0, AL.mult, AL.add)
            nc.vector.tensor_tensor(den[:], u1S, mk1[:], AL.add)
            nc.vector.reciprocal(rec[:], den[:])
            nc.vector.tensor_tensor(ib[:], px[:], rec[:], AL.mult)
            nc.vector.tensor_tensor(ib[:], ib[:], mk[:], AL.mult)
            # ol_c = where(c>0, min(ol, u2/c), ol)
            nc.vector.tensor_scalar(cmk[:], cS, 0.0, None, AL.is_gt)
            nc.vector.tensor_scalar(cm1[:], cmk[:], -1.0, 1.0, AL.mult, AL.add)
            nc.vector.tensor_tensor(csf[:], cS, cm1[:], AL.add)
            nc.vector.reciprocal(crc[:], csf[:])
            nc.vector.tensor_tensor(vv[:], u2S, crc[:], AL.mult)
            nc.vector.tensor_tensor(mn[:], ol[:], vv[:], AL.min)
            nc.vector.tensor_tensor(dd[:], mn[:], ol[:], AL.subtract)
            nc.vector.tensor_tensor(dd[:], dd[:], cmk[:], AL.mult)
            nc.vector.tensor_tensor(olc[:], dd[:], ol[:], AL.add)
            # f = 1 - oo - oogw - ol_c
            nc.vector.tensor_tensor(f1[:], oo[:], oogw[:], AL.add)
            nc.vector.tensor_tensor(f1[:], f1[:], olc[:], AL.add)
            nc.vector.tensor_scalar(ff[:], f1[:], -1.0, 1.0, AL.mult, AL.add)

            nc.sync.dma_start(outs["o_h"][:, :], hn[:])
            nc.sync.dma_start(outs["o_c"][:, :], cS)
            nc.sync.dma_start(outs["o_l"][:, :], ln[:])
            nc.sync.dma_start(outs["o_lc"][:, :], lcn[:])
            nc.sync.dma_start(outs["o_bp"][:, :], px[:])
            nc.sync.dma_start(outs["o_gw"][:, :], gwn[:])
            nc.sync.dma_start(outs["o_ib"][:, :], ib[:])
            nc.sync.dma_start(outs["o_oo"][:, :], oo[:])
            nc.sync.dma_start(outs["o_ol"][:, :], ol[:])
            nc.sync.dma_start(outs["o_olc"][:, :], olc[:])
            nc.sync.dma_start(outs["o_f"][:, :], ff[:])
            nc.sync.dma_start(outs["o_oogw"][:, :], oogw[:])

            # y_obs partial reductions (sum, sum of squares)
            yb = pool.tile([P, YCOLS], F32, name="yb")
            ysq = pool.tile([P, YCOLS], F32, name="ysq")
            yr = pool.tile([P, 2], F32, name="yr")
            nc.sync.dma_start(yb[:], yd[:, :])
            nc.vector.tensor_reduce(yr[:, 0:1], yb[:], mybir.AxisListType.X,
                                    AL.add)
            nc.vector.tensor_tensor(ysq[:], yb[:], yb[:], AL.mult)
            nc.vector.tensor_reduce(yr[:, 1:2], ysq[:], mybir.AxisListType.X,
                                    AL.add)
            nc.sync.dma_start(yred[:, :], yr[:])
    nc.compile()
    return nc


def _prep_inputs(u1, u2, y_obs):
    """Host-side re-layout into per-core padded chunk-lane arrays."""
    u1p = np.concatenate([np.zeros(W, np.float32), u1])
    u2p = np.concatenate([np.zeros(W, np.float32), u2])
    k_local = (np.arange(P)[:, None, None] * F + np.arange(F)[None, None, :])
    sig = np.arange(NSTEP)[None, :, None]
    in_maps = []
    for core in range(NCORE):
        idx = (core * P * F + k_local) * S + sig        # [P, NSTEP, F]
        a1 = u1p[idx.reshape(P, -1)]
        a2 = u2p[idx.reshape(P, -1)]
        yc = y_obs[core * CORE_T:(core + 1) * CORE_T].reshape(P, YCOLS)
        in_maps.append({"u1p": np.ascontiguousarray(a1),
                        "u2p": np.ascontiguousarray(a2),
                        "yp": np.ascontiguousarray(yc)})
    return in_maps


def _unlayout(arr_by_core):
    """[NCORE][P, S*F] chunk-lane layout -> (T,) time order."""
    parts = []
    for core in range(NCORE):
        a = arr_by_core[core].reshape(P, S, F).transpose(0, 2, 1)
        parts.append(a.reshape(-1))
    return np.concatenate(parts)


def _scalars(inp):
    f32 = np.float32
    g = {k: f32(np.asarray(inp[k]).reshape(-1)[0]) for k in
         ["weight_r_yom", "weight_r_yom_gw", "weight_r_ylm", "weight_r_yfm",
          "weight_b1_yom", "weight_b1_yom_gw", "weight_b2_ylm", "theltaC",
          "bias_b0_yom", "bias_b0_yom_gw", "bias_b0_ylm", "p_mean", "p_std"]}
    e_yom = np.exp(g["weight_r_yom"])
    e_gw = np.exp(g["weight_r_yom_gw"])
    e_ylm = np.exp(g["weight_r_ylm"])
    e_yfm = np.exp(g["weight_r_yfm"])
    den = e_yom + e_gw + e_ylm + e_yfm
    ML, SL = f32(2.9086), f32(1.898)
    mo, so = g["p_mean"], g["p_std"]
    sc = {}
    sc["oo1"] = float(f32(e_yom / den))
    sc["gw1"] = float(f32(e_gw / den))
    sc["ol1"] = float(f32(e_ylm / den))
    sc["expC"] = float(np.exp(g["theltaC"]))
    sc["A1"] = float(f32(g["weight_b1_yom"] / so))
    sc["B1"] = float(f32(g["bias_b0_yom"] - mo * g["weight_b1_yom"] / so))
    sc["A2"] = float(f32(g["weight_b1_yom_gw"] / so))
    sc["B2"] = float(f32(g["bias_b0_yom_gw"] - mo * g["weight_b1_yom_gw"] / so))
    sc["A3"] = float(f32(g["weight_b2_ylm"] / SL))
    sc["B3"] = float(f32(g["bias_b0_ylm"] - ML * g["weight_b2_ylm"] / SL))

    # quadratic fit of s2(c) = sigmoid(A2*c + B2) over the reachable state
    # range, folded into the cubic P(c) = c*(1 - gw1*s2(c)) in factored form.
    cmax = sc["expC"] + 1.5
    cg = np.linspace(-0.1, cmax, 4001)
    z = sc["A2"] * cg + sc["B2"]
    s2 = 1.0 / (1.0 + np.exp(-z))
    q2, q1, q0 = np.polyfit(cg, s2, 2)
    fit_err = np.abs(q0 + q1 * cg + q2 * cg * cg - s2).max()
    assert fit_err < 2e-6, f"s2 quadratic fit error too large: {fit_err}"
    K1 = 1.0 - sc["gw1"] * q0
    K2 = -sc["gw1"] * q1
    K3 = -sc["gw1"] * q2
    assert abs(K3) > 1e-12
    sc["alpha"] = float(np.float32(K2 / K3))
    sc["beta"] = float(np.float32(K1 / K3))
    sc["p3"] = float(np.float32(K3))
    return sc


def kernel(**inputs):
    x = np.asarray(inputs["x"], np.float32)
    y_obs = np.asarray(inputs["y_obs"], np.float32).reshape(-1)
    u1 = np.ascontiguousarray(x[:, 0, 0])
    u2 = np.ascontiguousarray(x[:, 0, 1])
    sc = _scalars(inputs)

    key = tuple(sorted(sc.items()))
    if key not in _cache:
        _cache[key] = _build_program(sc)
    nc = _cache[key]

    in_maps = _prep_inputs(u1, u2, y_obs)
    from concourse.bass_utils import run_bass_kernel_spmd
    res = run_bass_kernel_spmd(nc, in_maps, core_ids=list(range(NCORE)))
    r = res.results

    out = {nm: _unlayout([r[c][nm] for c in range(NCORE)]).reshape(T, 1)
           for nm in OUT_NAMES}

    # obsstd: combine per-core partial sums, subtract the head [0, SPIN)
    s1 = sum(float(r[c]["yred"][:, 0].sum(dtype=np.float64)) for c in range(4))
    s2 = sum(float(r[c]["yred"][:, 1].sum(dtype=np.float64)) for c in range(4))
    head = y_obs[:SPIN].astype(np.float64)
    s1 -= float(head.sum())
    s2 -= float((head * head).sum())
    n = TRAIN - SPIN
    var = (s2 - s1 * s1 / n) / (n - 1)
    obsstd = np.float32(np.sqrt(max(var, 0.0)))

    obs = np.full((T, 1), obsstd, np.float32)
    h_nout = np.concatenate([out["o_h"], obs], axis=1)
    return (out["o_h"], out["o_c"], out["o_l"], out["o_lc"], out["o_bp"],
            out["o_gw"], out["o_ib"], out["o_oo"], out["o_ol"], out["o_olc"],
            out["o_f"], out["o_oogw"], h_nout, obs)


# revision 3
# speedup vs baseline: 1.1433x; 1.1433x over previous
"""Trainium2 Bass kernel for nn_MCPBRNN_Generic_PETconstraint_Two_VariantOutputGate_BYPASSM0.

Strategy
--------
The model is a scalar-state (H=1) nonlinear recurrence over T=524288 steps.
Every per-step output is a pure elementwise function of the PRE-update state
c[t] and the inputs u1[t], u2[t], so the kernel has two phases:

  Phase A: compute the state trajectory c[t]. The map c -> F(c, t) is a strong
    contraction (|dF/dc| ~ 0.65), so the sequence is split into
    8 cores x 128 partitions x F lanes = 4096 independent chunks of S steps;
    each chunk starts from c=0 and runs W warm-up steps through the preceding
    inputs, which converges the state to fp32 noise (measured: W=32 reaches
    ~5e-7 worst-case; W=40 used for margin).  Per step the update is

      c1 = P(c) - oo1*sig(A1*c+B1)*c - min(ol_t*c, u2_t) - max(expC-u1_t, c) + expC
         = P(c) + y1 + relu(u2_t - ol_t*c) - relu(c - (expC-u1_t)) + (u1_t - u2_t)

    where P(c) = c*(1 - gw1*s2(c)) with the tiny-argument sigmoid s2 replaced
    by a host-fitted quadratic (max fit error ~1e-7, verified at runtime),
    factored as p3*((c+alpha)*c+beta)*c so each step is 3 ACT + 7 DVE + 3 Pool
    instructions.

  Phase B: recompute all 14 outputs from the stored c[t] with fully vectorized
    ops + DMA out.

Inputs are re-laid-out on the host so each SBUF column block [128, F] holds
one time step for 128*F chunks (chunk k <-> (partition p, lane f), k = p*F+f).
"""
import numpy as np

import concourse.bacc as bacc
import concourse.tile as tile
import concourse.mybir as mybir

P = 128
NCORE = 8
T = 524288
F = 32            # free-dim lanes per partition
S = 16            # steps stored per chunk;  P*F*S*NCORE == T
W = 32            # warm-up steps
NSTEP = W + S
COLS = NSTEP * F
CORE_T = P * F * S          # 65536 time steps handled per core
YCOLS = CORE_T // P         # y_obs columns per core (512)
SPIN = 365
TRAIN = 262144

AL = mybir.AluOpType
AF = mybir.ActivationFunctionType
F32 = mybir.dt.float32

OUT_NAMES = ["o_h", "o_c", "o_l", "o_lc", "o_bp", "o_gw",
             "o_ib", "o_oo", "o_ol", "o_olc", "o_f", "o_oogw"]

_cache = {}


def _build_program(sc):
    """sc: dict of python-float scalars baked into the program."""
    nc = bacc.Bacc("TRN2", target_bir_lowering=False, debug=False,
                   num_devices=NCORE)
    u1d = nc.dram_tensor("u1p", [P, COLS], F32, kind="ExternalInput").ap()
    u2d = nc.dram_tensor("u2p", [P, COLS], F32, kind="ExternalInput").ap()
    yd = nc.dram_tensor("yp", [P, YCOLS], F32, kind="ExternalInput").ap()
    outs = {nm: nc.dram_tensor(nm, [P, S * F], F32, kind="ExternalOutput").ap()
            for nm in OUT_NAMES}
    yred = nc.dram_tensor("yred", [P, 2], F32, kind="ExternalOutput").ap()

    with tile.TileContext(nc) as tc:
        with tc.tile_pool(name="main", bufs=1) as pool:
            # persistent buffers
            u1b = pool.tile([P, COLS], F32, name="u1b")
            u2b = pool.tile([P, COLS], F32, name="u2b")
            s3n = pool.tile([P, COLS], F32, name="s3n")   # -ol1*sigmoid(A3*u2+B3)
            qnb = pool.tile([P, COLS], F32, name="qnb")   # u1 - expC
            w12 = pool.tile([P, COLS], F32, name="w12")   # u1 - u2
            cb = pool.tile([P, COLS + F], F32, name="cb")
            # bias constants for ACT (bias must be an AP for non-Copy funcs)
            bB1 = pool.tile([P, 1], F32, name="bB1")
            bB3 = pool.tile([P, 1], F32, name="bB3")
            bB2 = pool.tile([P, 1], F32, name="bB2")
            bz = pool.tile([P, 1], F32, name="bz")
            nc.vector.memset(bB1[:], sc["B1"])
            nc.vector.memset(bB3[:], sc["B3"])
            nc.vector.memset(bB2[:], sc["B2"])
            nc.vector.memset(bz[:], 0.0)

            nc.sync.dma_start(u1b[:], u1d[:, :])
            nc.sync.dma_start(u2b[:], u2d[:, :])

            # big precomputes
            nc.scalar.activation(s3n[:], u2b[:], AF.Sigmoid, bias=bB3[:, 0:1],
                                 scale=sc["A3"])
            nc.vector.tensor_scalar(s3n[:], s3n[:], -sc["ol1"], None, AL.mult)
            nc.vector.tensor_scalar(qnb[:], u1b[:], -sc["expC"], None, AL.add)
            nc.vector.scalar_tensor_tensor(w12[:], u2b[:], -1.0, u1b[:],
                                           AL.mult, AL.add)
            nc.vector.memset(cb[:, 0:F], 0.0)

            # phase A temporaries
            names = ["s1", "e1", "e2", "y1", "t1", "t2", "aam", "p1", "cq",
                     "p2", "r2", "pq"]
            tmp = {nm: pool.tile([P, F], F32, name=f"pa_{nm}") for nm in names}

            for step in range(NSTEP):
                lo, hi = step * F, (step + 1) * F
                c = cb[:, lo:hi]
                s1, e1, e2, y1, t1, t2 = (tmp[n] for n in
                                          ("s1", "e1", "e2", "y1", "t1", "t2"))
                aam, p1, cq, p2, r2, pq = (tmp[n] for n in
                                           ("aam", "p1", "cq", "p2", "r2", "pq"))
                nc.gpsimd.tensor_tensor(cq[:], c, qnb[:, lo:hi], AL.add)
                nc.gpsimd.tensor_tensor(aam[:], s3n[:, lo:hi], c, AL.mult)
                nc.scalar.activation(s1[:], c, AF.Sigmoid, bias=bB1[:, 0:1],
                                     scale=sc["A1"])
                nc.scalar.activation(r2[:], cq[:], AF.Relu, bias=bz[:, 0:1],
                                     scale=1.0)
                nc.gpsimd.tensor_tensor(p1[:], aam[:], u2b[:, lo:hi], AL.add)
                nc.vector.scalar_tensor_tensor(e1[:], c, sc["alpha"], c,
                                               AL.add, AL.mult)
                nc.vector.scalar_tensor_tensor(e2[:], e1[:], sc["beta"], c,
                                               AL.add, AL.mult)
                nc.scalar.activation(p2[:], p1[:], AF.Relu, bias=bz[:, 0:1],
                                     scale=1.0)
                nc.vector.scalar_tensor_tensor(y1[:], s1[:], -sc["oo1"], c,
                                               AL.mult, AL.mult)
                nc.vector.scalar_tensor_tensor(t1[:], e2[:], sc["p3"], y1[:],
                                               AL.mult, AL.add)
                nc.vector.scalar_tensor_tensor(pq[:], r2[:], -1.0, p2[:],
                                               AL.mult, AL.add)
                nc.gpsimd.tensor_tensor(t2[:], t1[:], w12[:, lo:hi], AL.add)
                nc.gpsimd.tensor_tensor(cb[:, hi:hi + F], t2[:], pq[:], AL.add)

            # ---- phase B ----
            st = W * F           # start col of the stored region
            cS = cb[:, st:COLS]
            u1S = u1b[:, st:COLS]
            u2S = u2b[:, st:COLS]
            qnS = qnb[:, st:COLS]
            s3S = s3n[:, st:COLS]
            NB = S * F

            def bt(nm):
                return pool.tile([P, NB], F32, name=f"pb_{nm}")

            s1b = bt("s1b"); s2b = bt("s2b"); oo = bt("oo"); oogw = bt("oogw")
            ol = bt("ol"); pxs = bt("pxs"); px = bt("px"); hn = bt("hn")
            ln = bt("ln"); lcn = bt("lcn"); gwn = bt("gwn")
            mk = bt("mk"); mk1 = bt("mk1"); den = bt("den"); rec = bt("rec")
            ib = bt("ib"); cmk = bt("cmk"); cm1 = bt("cm1"); csf = bt("csf")
            crc = bt("crc"); vv = bt("vv"); mn = bt("mn"); dd = bt("dd")
            olc = bt("olc"); f1 = bt("f1"); ff = bt("ff"); t0 = bt("t0")

            nc.scalar.activation(s1b[:], cS, AF.Sigmoid, bias=bB1[:, 0:1],
                                 scale=sc["A1"])
            nc.scalar.activation(s2b[:], cS, AF.Sigmoid, bias=bB2[:, 0:1],
                                 scale=sc["A2"])
            nc.vector.tensor_scalar(oo[:], s1b[:], sc["oo1"], None, AL.mult)
            nc.vector.tensor_scalar(oogw[:], s2b[:], sc["gw1"], None, AL.mult)
            nc.vector.tensor_scalar(ol[:], s3S, -1.0, None, AL.mult)
            nc.vector.tensor_tensor(pxs[:], cS, qnS, AL.add)
            nc.scalar.activation(px[:], pxs[:], AF.Relu, bias=bz[:, 0:1], scale=1.0)
            nc.gpsimd.tensor_tensor(t0[:], oo[:], cS, AL.mult)
            nc.gpsimd.tensor_tensor(hn[:], t0[:], px[:], AL.add)
            nc.gpsimd.tensor_tensor(ln[:], ol[:], cS, AL.mult)
            nc.vector.tensor_tensor(lcn[:], ln[:], u2S, AL.min)
            nc.gpsimd.tensor_tensor(gwn[:], oogw[:], cS, AL.mult)
            # ib = where(u1>0, px/u1, 0)
            nc.vector.tensor_scalar(mk[:], u1S, 0.0, None, AL.is_gt)
            nc.vector.tensor_scalar(mk1[:], mk[:], -1.0, 1.0, AL.mult, AL.add)
            nc.vector.tensor_tensor(den[:], u1S, mk1[:], AL.add)
            nc.vector.reciprocal(rec[:], den[:])
            nc.vector.tensor_tensor(ib[:], px[:], rec[:], AL.mult)
            nc.vector.tensor_tensor(ib[:], ib[:], mk[:], AL.mult)
            # ol_c = where(c>0, min(ol, u2/c), ol)
            nc.vector.tensor_scalar(cmk[:], cS, 0.0, None, AL.is_gt)
            nc.vector.tensor_scalar(cm1[:], cmk[:], -1.0, 1.0, AL.mult, AL.add)
            nc.vector.tensor_tensor(csf[:], cS, cm1[:], AL.add)
            nc.vector.reciprocal(crc[:], csf[:])
            nc.vector.tensor_tensor(vv[:], u2S, crc[:], AL.mult)
            nc.vector.tensor_tensor(mn[:], ol[:], vv[:], AL.min)
            nc.vector.tensor_tensor(dd[:], mn[:], ol[:], AL.subtract)
            nc.vector.tensor_tensor(dd[:], dd[:], cmk[:], AL.mult)
            nc.vector.tensor_tensor(olc[:], dd[:], ol[:], AL.add)
            # f = 1 - oo - oogw - ol_c
            nc.gpsimd.tensor_tensor(f1[:], oo[:], oogw[:], AL.add)
            nc.gpsimd.tensor_tensor(f1[:], f1[:], olc[:], AL.add)
            nc.vector.tensor_scalar(ff[:], f1[:], -1.0, 1.0, AL.mult, AL.add)

            nc.sync.dma_start(outs["o_h"][:, :], hn[:])
            nc.sync.dma_start(outs["o_c"][:, :], cS)
            nc.sync.dma_start(outs["o_l"][:, :], ln[:])
            nc.sync.dma_start(outs["o_lc"][:, :], lcn[:])
            nc.sync.dma_start(outs["o_bp"][:, :], px[:])
            nc.sync.dma_start(outs["o_gw"][:, :], gwn[:])
            nc.sync.dma_start(outs["o_ib"][:, :], ib[:])
            nc.sync.dma_start(outs["o_oo"][:, :], oo[:])
            nc.sync.dma_start(outs["o_ol"][:, :], ol[:])
            nc.sync.dma_start(outs["o_olc"][:, :], olc[:])
            nc.sync.dma_start(outs["o_f"][:, :], ff[:])
            nc.sync.dma_start(outs["o_oogw"][:, :], oogw[:])

            # y_obs partial reductions (sum, sum of squares)
            yb = pool.tile([P, YCOLS], F32, name="yb")
            ysq = pool.tile([P, YCOLS], F32, name="ysq")
            yr = pool.tile([P, 2], F32, name="yr")
            nc.sync.dma_start(yb[:], yd[:, :])
            nc.vector.tensor_reduce(yr[:, 0:1], yb[:], mybir.AxisListType.X,
                                    AL.add)
            nc.vector.tensor_tensor(ysq[:], yb[:], yb[:], AL.mult)
            nc.vector.tensor_reduce(yr[:, 1:2], ysq[:], mybir.AxisListType.X,
                                    AL.add)
            nc.sync.dma_start(yred[:, :], yr[:])
    nc.compile()
    return nc


def _prep_inputs(u1, u2, y_obs):
    """Host-side re-layout into per-core padded chunk-lane arrays."""
    u1p = np.concatenate([np.zeros(W, np.float32), u1])
    u2p = np.concatenate([np.zeros(W, np.float32), u2])
    k_local = (np.arange(P)[:, None, None] * F + np.arange(F)[None, None, :])
    sig = np.arange(NSTEP)[None, :, None]
    in_maps = []
    for core in range(NCORE):
        idx = (core * P * F + k_local) * S + sig        # [P, NSTEP, F]
        a1 = u1p[idx.reshape(P, -1)]
        a2 = u2p[idx.reshape(P, -1)]
        yc = y_obs[core * CORE_T:(core + 1) * CORE_T].reshape(P, YCOLS)
        in_maps.append({"u1p": np.ascontiguousarray(a1),
                        "u2p": np.ascontiguousarray(a2),
                        "yp": np.ascontiguousarray(yc)})
    return in_maps


def _unlayout(arr_by_core):
    """[NCORE][P, S*F] chunk-lane layout -> (T,) time order."""
    parts = []
    for core in range(NCORE):
        a = arr_by_core[core].reshape(P, S, F).transpose(0, 2, 1)
        parts.append(a.reshape(-1))
    return np.concatenate(parts)


def _scalars(inp):
    f32 = np.float32
    g = {k: f32(np.asarray(inp[k]).reshape(-1)[0]) for k in
         ["weight_r_yom", "weight_r_yom_gw", "weight_r_ylm", "weight_r_yfm",
          "weight_b1_yom", "weight_b1_yom_gw", "weight_b2_ylm", "theltaC",
          "bias_b0_yom", "bias_b0_yom_gw", "bias_b0_ylm", "p_mean", "p_std"]}
    e_yom = np.exp(g["weight_r_yom"])
    e_gw = np.exp(g["weight_r_yom_gw"])
    e_ylm = np.exp(g["weight_r_ylm"])
    e_yfm = np.exp(g["weight_r_yfm"])
    den = e_yom + e_gw + e_ylm + e_yfm
    ML, SL = f32(2.9086), f32(1.898)
    mo, so = g["p_mean"], g["p_std"]
    sc = {}
    sc["oo1"] = float(f32(e_yom / den))
    sc["gw1"] = float(f32(e_gw / den))
    sc["ol1"] = float(f32(e_ylm / den))
    sc["expC"] = float(np.exp(g["theltaC"]))
    sc["A1"] = float(f32(g["weight_b1_yom"] / so))
    sc["B1"] = float(f32(g["bias_b0_yom"] - mo * g["weight_b1_yom"] / so))
    sc["A2"] = float(f32(g["weight_b1_yom_gw"] / so))
    sc["B2"] = float(f32(g["bias_b0_yom_gw"] - mo * g["weight_b1_yom_gw"] / so))
    sc["A3"] = float(f32(g["weight_b2_ylm"] / SL))
    sc["B3"] = float(f32(g["bias_b0_ylm"] - ML * g["weight_b2_ylm"] / SL))

    # quadratic fit of s2(c) = sigmoid(A2*c + B2) over the reachable state
    # range, folded into the cubic P(c) = c*(1 - gw1*s2(c)) in factored form.
    cmax = sc["expC"] + 1.5
    cg = np.linspace(-0.1, cmax, 4001)
    z = sc["A2"] * cg + sc["B2"]
    s2 = 1.0 / (1.0 + np.exp(-z))
    q2, q1, q0 = np.polyfit(cg, s2, 2)
    fit_err = np.abs(q0 + q1 * cg + q2 * cg * cg - s2).max()
    assert fit_err < 2e-6, f"s2 quadratic fit error too large: {fit_err}"
    K1 = 1.0 - sc["gw1"] * q0
    K2 = -sc["gw1"] * q1
    K3 = -sc["gw1"] * q2
    assert abs(K3) > 1e-12
    sc["alpha"] = float(np.float32(K2 / K3))
    sc["beta"] = float(np.float32(K1 / K3))
    sc["p3"] = float(np.float32(K3))
    return sc


def kernel(**inputs):
    x = np.asarray(inputs["x"], np.float32)
    y_obs = np.asarray(inputs["y_obs"], np.float32).reshape(-1)
    u1 = np.ascontiguousarray(x[:, 0, 0])
    u2 = np.ascontiguousarray(x[:, 0, 1])
    sc = _scalars(inputs)

    key = tuple(sorted(sc.items()))
    if key not in _cache:
        _cache[key] = _build_program(sc)
    nc = _cache[key]

    in_maps = _prep_inputs(u1, u2, y_obs)
    from concourse.bass_utils import run_bass_kernel_spmd
    res = run_bass_kernel_spmd(nc, in_maps, core_ids=list(range(NCORE)))
    r = res.results

    out = {nm: _unlayout([r[c][nm] for c in range(NCORE)]).reshape(T, 1)
           for nm in OUT_NAMES}

    # obsstd: combine per-core partial sums, subtract the head [0, SPIN)
    s1 = sum(float(r[c]["yred"][:, 0].sum(dtype=np.float64)) for c in range(4))
    s2 = sum(float(r[c]["yred"][:, 1].sum(dtype=np.float64)) for c in range(4))
    head = y_obs[:SPIN].astype(np.float64)
    s1 -= float(head.sum())
    s2 -= float((head * head).sum())
    n = TRAIN - SPIN
    var = (s2 - s1 * s1 / n) / (n - 1)
    obsstd = np.float32(np.sqrt(max(var, 0.0)))

    obs = np.full((T, 1), obsstd, np.float32)
    h_nout = np.concatenate([out["o_h"], obs], axis=1)
    return (out["o_h"], out["o_c"], out["o_l"], out["o_lc"], out["o_bp"],
            out["o_gw"], out["o_ib"], out["o_oo"], out["o_ol"], out["o_olc"],
            out["o_f"], out["o_oogw"], h_nout, obs)


# revision 4
# speedup vs baseline: 1.1593x; 1.0140x over previous
"""Trainium2 Bass kernel for nn_MCPBRNN_Generic_PETconstraint_Two_VariantOutputGate_BYPASSM0.

Strategy
--------
The model is a scalar-state (H=1) nonlinear recurrence over T=524288 steps.
Every per-step output is a pure elementwise function of the PRE-update state
c[t] and the inputs u1[t], u2[t], so the kernel has two phases:

  Phase A: compute the state trajectory c[t]. The map c -> F(c, t) is a strong
    contraction (|dF/dc| ~ 0.65), so the sequence is split into
    8 cores x 128 partitions x F lanes = 4096 independent chunks of S steps;
    each chunk starts from c=0 and runs W warm-up steps through the preceding
    inputs, which converges the state to fp32 noise (measured: W=32 reaches
    ~5e-7 worst-case; W=40 used for margin).  Per step the update is

      c1 = P(c) - oo1*sig(A1*c+B1)*c - min(ol_t*c, u2_t) - max(expC-u1_t, c) + expC
         = P(c) + y1 + relu(u2_t - ol_t*c) - relu(c - (expC-u1_t)) + (u1_t - u2_t)

    where P(c) = c*(1 - gw1*s2(c)) with the tiny-argument sigmoid s2 replaced
    by a host-fitted quadratic (max fit error ~1e-7, verified at runtime),
    factored as p3*((c+alpha)*c+beta)*c so each step is 3 ACT + 7 DVE + 3 Pool
    instructions.

  Phase B: recompute all 14 outputs from the stored c[t] with fully vectorized
    ops + DMA out.

Inputs are re-laid-out on the host so each SBUF column block [128, F] holds
one time step for 128*F chunks (chunk k <-> (partition p, lane f), k = p*F+f).
"""
import numpy as np

import concourse.bacc as bacc
import concourse.tile as tile
import concourse.mybir as mybir

P = 128
NCORE = 8
T = 524288
F = 32            # free-dim lanes per partition
S = 16            # steps stored per chunk;  P*F*S*NCORE == T
W = 32            # warm-up steps
NSTEP = W + S
COLS = NSTEP * F
CORE_T = P * F * S          # 65536 time steps handled per core
YCOLS = CORE_T // P         # y_obs columns per core (512)
SPIN = 365
TRAIN = 262144

AL = mybir.AluOpType
AF = mybir.ActivationFunctionType
F32 = mybir.dt.float32

OUT_NAMES = ["o_h", "o_c", "o_l", "o_lc", "o_bp", "o_gw",
             "o_ib", "o_oo", "o_ol", "o_olc", "o_f", "o_oogw"]

_cache = {}


def _build_program(sc):
    """sc: dict of python-float scalars baked into the program."""
    nc = bacc.Bacc("TRN2", target_bir_lowering=False, debug=False,
                   num_devices=NCORE)
    u1d = nc.dram_tensor("u1p", [P, COLS], F32, kind="ExternalInput").ap()
    u2d = nc.dram_tensor("u2p", [P, COLS], F32, kind="ExternalInput").ap()
    yd = nc.dram_tensor("yp", [P, YCOLS], F32, kind="ExternalInput").ap()
    outs = {nm: nc.dram_tensor(nm, [P, S * F], F32, kind="ExternalOutput").ap()
            for nm in OUT_NAMES}
    yred = nc.dram_tensor("yred", [P, 2], F32, kind="ExternalOutput").ap()

    with tile.TileContext(nc) as tc:
        with tc.tile_pool(name="main", bufs=1) as pool:
            # persistent buffers
            u1b = pool.tile([P, COLS], F32, name="u1b")
            u2b = pool.tile([P, COLS], F32, name="u2b")
            s3n = pool.tile([P, COLS], F32, name="s3n")   # -ol1*sigmoid(A3*u2+B3)
            qnb = pool.tile([P, COLS], F32, name="qnb")   # u1 - expC
            w12 = pool.tile([P, COLS], F32, name="w12")   # u1 - u2
            cb = pool.tile([P, COLS + F], F32, name="cb")
            # bias constants for ACT (bias must be an AP for non-Copy funcs)
            bB1 = pool.tile([P, 1], F32, name="bB1")
            bB3 = pool.tile([P, 1], F32, name="bB3")
            bB2 = pool.tile([P, 1], F32, name="bB2")
            bz = pool.tile([P, 1], F32, name="bz")
            nc.vector.memset(bB1[:], sc["B1"])
            nc.vector.memset(bB3[:], sc["B3"])
            nc.vector.memset(bB2[:], sc["B2"])
            nc.vector.memset(bz[:], 0.0)

            nc.sync.dma_start(u1b[:], u1d[:, :])
            nc.sync.dma_start(u2b[:], u2d[:, :])

            # big precomputes
            nc.scalar.activation(s3n[:], u2b[:], AF.Sigmoid, bias=bB3[:, 0:1],
                                 scale=sc["A3"])
            nc.vector.tensor_scalar(s3n[:], s3n[:], -sc["ol1"], None, AL.mult)
            nc.vector.tensor_scalar(qnb[:], u1b[:], -sc["expC"], None, AL.add)
            nc.vector.scalar_tensor_tensor(w12[:], u2b[:], -1.0, u1b[:],
                                           AL.mult, AL.add)
            nc.vector.memset(cb[:, 0:F], 0.0)

            # phase A temporaries
            names = ["s1", "e1", "e2", "y1", "t1", "t2", "aam", "p1", "cq",
                     "p2", "r2", "pq"]
            tmp = {nm: pool.tile([P, F], F32, name=f"pa_{nm}") for nm in names}

            for step in range(NSTEP):
                lo, hi = step * F, (step + 1) * F
                c = cb[:, lo:hi]
                s1, e1, e2, y1, t1, t2 = (tmp[n] for n in
                                          ("s1", "e1", "e2", "y1", "t1", "t2"))
                aam, p1, cq, p2, r2, pq = (tmp[n] for n in
                                           ("aam", "p1", "cq", "p2", "r2", "pq"))
                nc.gpsimd.tensor_tensor(aam[:], s3n[:, lo:hi], c, AL.mult)
                nc.scalar.activation(s1[:], c, AF.Sigmoid, bias=bB1[:, 0:1],
                                     scale=sc["A1"])
                nc.gpsimd.tensor_tensor(p1[:], aam[:], u2b[:, lo:hi], AL.add)
                nc.gpsimd.tensor_tensor(cq[:], c, qnb[:, lo:hi], AL.add)
                nc.vector.scalar_tensor_tensor(e1[:], c, sc["alpha"], c,
                                               AL.add, AL.mult)
                nc.scalar.activation(p2[:], p1[:], AF.Relu, bias=bz[:, 0:1],
                                     scale=1.0)
                nc.scalar.activation(r2[:], cq[:], AF.Relu, bias=bz[:, 0:1],
                                     scale=1.0)
                nc.vector.scalar_tensor_tensor(y1[:], s1[:], -sc["oo1"], c,
                                               AL.mult, AL.mult)
                nc.vector.scalar_tensor_tensor(t1[:], e1[:], sc["p3"], y1[:],
                                               AL.mult, AL.add)
                nc.vector.scalar_tensor_tensor(pq[:], r2[:], -1.0, p2[:],
                                               AL.mult, AL.add)
                nc.gpsimd.tensor_tensor(t2[:], t1[:], w12[:, lo:hi], AL.add)
                nc.gpsimd.tensor_tensor(cb[:, hi:hi + F], t2[:], pq[:], AL.add)

            # ---- phase B ----
            st = W * F           # start col of the stored region
            cS = cb[:, st:COLS]
            u1S = u1b[:, st:COLS]
            u2S = u2b[:, st:COLS]
            qnS = qnb[:, st:COLS]
            s3S = s3n[:, st:COLS]
            NB = S * F

            def bt(nm):
                return pool.tile([P, NB], F32, name=f"pb_{nm}")

            s1b = bt("s1b"); s2b = bt("s2b"); oo = bt("oo"); oogw = bt("oogw")
            ol = bt("ol"); pxs = bt("pxs"); px = bt("px"); hn = bt("hn")
            ln = bt("ln"); lcn = bt("lcn"); gwn = bt("gwn")
            mk = bt("mk"); mk1 = bt("mk1"); den = bt("den"); rec = bt("rec")
            ib = bt("ib"); cmk = bt("cmk"); cm1 = bt("cm1"); csf = bt("csf")
            crc = bt("crc"); vv = bt("vv"); mn = bt("mn"); dd = bt("dd")
            olc = bt("olc"); f1 = bt("f1"); ff = bt("ff"); t0 = bt("t0")

            nc.scalar.activation(s1b[:], cS, AF.Sigmoid, bias=bB1[:, 0:1],
                                 scale=sc["A1"])
            nc.scalar.activation(s2b[:], cS, AF.Sigmoid, bias=bB2[:, 0:1],
                                 scale=sc["A2"])
            nc.vector.tensor_scalar(oo[:], s1b[:], sc["oo1"], None, AL.mult)
            nc.vector.tensor_scalar(oogw[:], s2b[:], sc["gw1"], None, AL.mult)
            nc.vector.tensor_scalar(ol[:], s3S, -1.0, None, AL.mult)
            nc.vector.tensor_tensor(pxs[:], cS, qnS, AL.add)
            nc.scalar.activation(px[:], pxs[:], AF.Relu, bias=bz[:, 0:1], scale=1.0)
            nc.gpsimd.tensor_tensor(t0[:], oo[:], cS, AL.mult)
            nc.gpsimd.tensor_tensor(hn[:], t0[:], px[:], AL.add)
            nc.gpsimd.tensor_tensor(ln[:], ol[:], cS, AL.mult)
            nc.vector.tensor_tensor(lcn[:], ln[:], u2S, AL.min)
            nc.gpsimd.tensor_tensor(gwn[:], oogw[:], cS, AL.mult)
            # ib = where(u1>0, px/u1, 0)
            nc.vector.tensor_scalar(mk[:], u1S, 0.0, None, AL.is_gt)
            nc.vector.tensor_scalar(mk1[:], mk[:], -1.0, 1.0, AL.mult, AL.add)
            nc.vector.tensor_tensor(den[:], u1S, mk1[:], AL.add)
            nc.vector.reciprocal(rec[:], den[:])
            nc.vector.tensor_tensor(ib[:], px[:], rec[:], AL.mult)
            nc.vector.tensor_tensor(ib[:], ib[:], mk[:], AL.mult)
            # ol_c = where(c>0, min(ol, u2/c), ol)
            nc.vector.tensor_scalar(cmk[:], cS, 0.0, None, AL.is_gt)
            nc.vector.tensor_scalar(cm1[:], cmk[:], -1.0, 1.0, AL.mult, AL.add)
            nc.vector.tensor_tensor(csf[:], cS, cm1[:], AL.add)
            nc.vector.reciprocal(crc[:], csf[:])
            nc.vector.tensor_tensor(vv[:], u2S, crc[:], AL.mult)
            nc.vector.tensor_tensor(mn[:], ol[:], vv[:], AL.min)
            nc.vector.tensor_tensor(dd[:], mn[:], ol[:], AL.subtract)
            nc.vector.tensor_tensor(dd[:], dd[:], cmk[:], AL.mult)
            nc.vector.tensor_tensor(olc[:], dd[:], ol[:], AL.add)
            # f = 1 - oo - oogw - ol_c
            nc.gpsimd.tensor_tensor(f1[:], oo[:], oogw[:], AL.add)
            nc.gpsimd.tensor_tensor(f1[:], f1[:], olc[:], AL.add)
            nc.vector.tensor_scalar(ff[:], f1[:], -1.0, 1.0, AL.mult, AL.add)

            nc.sync.dma_start(outs["o_h"][:, :], hn[:])
            nc.sync.dma_start(outs["o_c"][:, :], cS)
            nc.sync.dma_start(outs["o_l"][:, :], ln[:])
            nc.sync.dma_start(outs["o_lc"][:, :], lcn[:])
            nc.sync.dma_start(outs["o_bp"][:, :], px[:])
            nc.sync.dma_start(outs["o_gw"][:, :], gwn[:])
            nc.sync.dma_start(outs["o_ib"][:, :], ib[:])
            nc.sync.dma_start(outs["o_oo"][:, :], oo[:])
            nc.sync.dma_start(outs["o_ol"][:, :], ol[:])
            nc.sync.dma_start(outs["o_olc"][:, :], olc[:])
            nc.sync.dma_start(outs["o_f"][:, :], ff[:])
            nc.sync.dma_start(outs["o_oogw"][:, :], oogw[:])

            # y_obs partial reductions (sum, sum of squares)
            yb = pool.tile([P, YCOLS], F32, name="yb")
            ysq = pool.tile([P, YCOLS], F32, name="ysq")
            yr = pool.tile([P, 2], F32, name="yr")
            nc.sync.dma_start(yb[:], yd[:, :])
            nc.vector.tensor_reduce(yr[:, 0:1], yb[:], mybir.AxisListType.X,
                                    AL.add)
            nc.vector.tensor_tensor(ysq[:], yb[:], yb[:], AL.mult)
            nc.vector.tensor_reduce(yr[:, 1:2], ysq[:], mybir.AxisListType.X,
                                    AL.add)
            nc.sync.dma_start(yred[:, :], yr[:])
    nc.compile()
    return nc


def _prep_inputs(u1, u2, y_obs):
    """Host-side re-layout into per-core padded chunk-lane arrays."""
    u1p = np.concatenate([np.zeros(W, np.float32), u1])
    u2p = np.concatenate([np.zeros(W, np.float32), u2])
    k_local = (np.arange(P)[:, None, None] * F + np.arange(F)[None, None, :])
    sig = np.arange(NSTEP)[None, :, None]
    in_maps = []
    for core in range(NCORE):
        idx = (core * P * F + k_local) * S + sig        # [P, NSTEP, F]
        a1 = u1p[idx.reshape(P, -1)]
        a2 = u2p[idx.reshape(P, -1)]
        yc = y_obs[core * CORE_T:(core + 1) * CORE_T].reshape(P, YCOLS)
        in_maps.append({"u1p": np.ascontiguousarray(a1),
                        "u2p": np.ascontiguousarray(a2),
                        "yp": np.ascontiguousarray(yc)})
    return in_maps


def _unlayout(arr_by_core):
    """[NCORE][P, S*F] chunk-lane layout -> (T,) time order."""
    parts = []
    for core in range(NCORE):
        a = arr_by_core[core].reshape(P, S, F).transpose(0, 2, 1)
        parts.append(a.reshape(-1))
    return np.concatenate(parts)


def _scalars(inp):
    f32 = np.float32
    g = {k: f32(np.asarray(inp[k]).reshape(-1)[0]) for k in
         ["weight_r_yom", "weight_r_yom_gw", "weight_r_ylm", "weight_r_yfm",
          "weight_b1_yom", "weight_b1_yom_gw", "weight_b2_ylm", "theltaC",
          "bias_b0_yom", "bias_b0_yom_gw", "bias_b0_ylm", "p_mean", "p_std"]}
    e_yom = np.exp(g["weight_r_yom"])
    e_gw = np.exp(g["weight_r_yom_gw"])
    e_ylm = np.exp(g["weight_r_ylm"])
    e_yfm = np.exp(g["weight_r_yfm"])
    den = e_yom + e_gw + e_ylm + e_yfm
    ML, SL = f32(2.9086), f32(1.898)
    mo, so = g["p_mean"], g["p_std"]
    sc = {}
    sc["oo1"] = float(f32(e_yom / den))
    sc["gw1"] = float(f32(e_gw / den))
    sc["ol1"] = float(f32(e_ylm / den))
    sc["expC"] = float(np.exp(g["theltaC"]))
    sc["A1"] = float(f32(g["weight_b1_yom"] / so))
    sc["B1"] = float(f32(g["bias_b0_yom"] - mo * g["weight_b1_yom"] / so))
    sc["A2"] = float(f32(g["weight_b1_yom_gw"] / so))
    sc["B2"] = float(f32(g["bias_b0_yom_gw"] - mo * g["weight_b1_yom_gw"] / so))
    sc["A3"] = float(f32(g["weight_b2_ylm"] / SL))
    sc["B3"] = float(f32(g["bias_b0_ylm"] - ML * g["weight_b2_ylm"] / SL))

    # quadratic fit of s2(c) = sigmoid(A2*c + B2) over the reachable state
    # range, folded into the cubic P(c) = c*(1 - gw1*s2(c)) in factored form.
    cmax = sc["expC"] + 1.3
    cg = np.linspace(-0.05, cmax, 4001)
    z = sc["A2"] * cg + sc["B2"]
    s2 = 1.0 / (1.0 + np.exp(-z))
    q1, q0 = np.polyfit(cg, s2, 1)
    fit_err = np.abs(q0 + q1 * cg - s2).max()
    # error in c1 per step ~ gw1*cmax*fit_err; contraction sum ~3x
    assert fit_err * sc["gw1"] * cmax * 3.0 < 2e-4,         f"s2 linear fit error too large: {fit_err}"
    K1 = 1.0 - sc["gw1"] * q0
    K2 = -sc["gw1"] * q1
    assert abs(K2) > 1e-12
    sc["alpha"] = float(np.float32(K1 / K2))
    sc["p3"] = float(np.float32(K2))
    return sc


def kernel(**inputs):
    x = np.asarray(inputs["x"], np.float32)
    y_obs = np.asarray(inputs["y_obs"], np.float32).reshape(-1)
    u1 = np.ascontiguousarray(x[:, 0, 0])
    u2 = np.ascontiguousarray(x[:, 0, 1])
    sc = _scalars(inputs)

    key = tuple(sorted(sc.items()))
    if key not in _cache:
        _cache[key] = _build_program(sc)
    nc = _cache[key]

    in_maps = _prep_inputs(u1, u2, y_obs)
    from concourse.bass_utils import run_bass_kernel_spmd
    res = run_bass_kernel_spmd(nc, in_maps, core_ids=list(range(NCORE)))
    r = res.results

    out = {nm: _unlayout([r[c][nm] for c in range(NCORE)]).reshape(T, 1)
           for nm in OUT_NAMES}

    # obsstd: combine per-core partial sums, subtract the head [0, SPIN)
    s1 = sum(float(r[c]["yred"][:, 0].sum(dtype=np.float64)) for c in range(4))
    s2 = sum(float(r[c]["yred"][:, 1].sum(dtype=np.float64)) for c in range(4))
    head = y_obs[:SPIN].astype(np.float64)
    s1 -= float(head.sum())
    s2 -= float((head * head).sum())
    n = TRAIN - SPIN
    var = (s2 - s1 * s1 / n) / (n - 1)
    obsstd = np.float32(np.sqrt(max(var, 0.0)))

    obs = np.full((T, 1), obsstd, np.float32)
    h_nout = np.concatenate([out["o_h"], obs], axis=1)
    return (out["o_h"], out["o_c"], out["o_l"], out["o_lc"], out["o_bp"],
            out["o_gw"], out["o_ib"], out["o_oo"], out["o_ol"], out["o_olc"],
            out["o_f"], out["o_oogw"], h_nout, obs)
